# revision 22
# baseline (speedup 1.0000x reference)
"""PrRoIPool2D (precise ROI pooling) Trainium2 kernel — 8-core SPMD.

Strategy ("fused banded sweep", v3):
  out[r,c,p,q] = sum_{h,w} F[b_r,c,h,w] * Iy[r,p,h] * Ix[r,q,w]
The (Iy ⊗ Ix) basis is banded: bin (r,p) touches only a ~4-row window of h.
Each core owns one feature batch.  Host packs a basis tensor B whose columns
are (r,p,q) output septets; for each 2-row h-chunk k the alive columns form
one contiguous interval [LO_k, HI_k).  The device runs one matmul per
(chunk, c-half, psum-bank-piece) with the features as stationary weights,
PSUM-accumulating straight into the final output columns.

v3+ over v2 (each measured on HW, min/median of 5 — run variance ~1.5us):
  * Slot assignment solved as an LP (per-core increasing slot maps, shared
    per-chunk staircase intervals, minimize total interval width) with slot
    slack NSLOT=288 > Rmax*7: interval columns drop ~30%.  Ties broken
    toward early bank retirement via per-sigma lower bounds (free).
  * No full-coverage chain fix: PSUM `start` clears the whole bank's
    has_written bits, so never-written slack columns just read back garbage
    that the host ignores.
  * Tent tails carrying <1% of a window's weight are trimmed (rel err
    2.7e-3 -> 4.9e-3, budget 2e-2): ~7% fewer basis columns.
  * The 16 zero pad rows (K 112->128) ARE shipped from DRAM: SDMA port
    swizzle gives partitions 112-127 only underloaded ports, so they cost
    no stream time, while K=128 matmuls are ~1.33x faster per column than
    K=112 (cost model claims K-independence; HW disagrees).
  * First two input DMA triggers spliced ahead of the framework's init
    barrier (data starts ~7.5us instead of ~8.3us).
  * Six input splits with a small final one; all triggers on sync in
    emission order (the Tile runtime throttles outstanding DMAs with a
    FIFO semaphore chain — cross-engine or too-fine triggers reorder or
    pace the wire and slow the stream; 7/8-way splits measured no better).
  * Output stored per retired PSUM bank (4 pieces), evacuation split
    vector (half 0) / scalar (half 1) so the final bank drains in one hop.
"""

import numpy as np
import ml_dtypes

POOLED = 7
SCALE = 0.5
N, C, H, W = 8, 256, 56, 56
NCORES = 8
CHUNK_H = 2
NCHUNK = H // CHUNK_H          # 28
KDIM = CHUNK_H * W             # 112 (payload K rows; DMAed)
KPAD = 128                     # PE-array K (pad rows zeroed on device)
BANK = 512                     # fp32 elements per PSUM bank
BF16 = ml_dtypes.bfloat16

_kernel_cache = {}
LAST_RESULTS = None            # BassKernelResults stash for test harnesses


def _tent_integral(start, end, n):
    i = np.arange(n, dtype=np.float64)
    a = np.clip(start[..., None] - i, -1.0, 1.0)
    b = np.clip(end[..., None] - i, -1.0, 1.0)

    def G(t):
        return np.where(t <= 0.0, 0.5 * (t + 1.0) ** 2, 1.0 - 0.5 * (1.0 - t) ** 2)

    return G(b) - G(a)


def _assign_slots_lp(spans, NSLOT, retire=()):
    """Per-core increasing slot maps minimizing total staircase width.

    spans[c] = [(a, b)] chunk spans per window, sorted by (lo, hi).
    Returns slots[c] (np.int64 arrays).  LP: vars sigma (slots), L_k, H_k;
    min sum(H-L) s.t. sigma increasing per core, L_k <= sigma(first alive),
    H_k >= sigma(last alive)+1.  Constraint matrix is network-like, so the
    relaxation is integral in practice; rounding is repaired monotone.
    """
    from scipy.optimize import linprog
    from scipy.sparse import lil_matrix

    NC = len(spans)
    ncore = [len(s) for s in spans]
    fk = [[None] * NCHUNK for _ in range(NC)]
    lk = [[None] * NCHUNK for _ in range(NC)]
    for c in range(NC):
        for j, (a, b) in enumerate(spans[c]):
            for k in range(a, b + 1):
                if fk[c][k] is None:
                    fk[c][k] = j
                lk[c][k] = j
    off = np.cumsum([0] + ncore)
    nS = off[-1]
    iL, iH = nS, nS + NCHUNK
    nv = nS + 2 * NCHUNK
    # bank-retirement: windows alive at chunk >= kf must sit at slot >= ms
    # so low PSUM banks retire (and store) while the stream still runs.
    # Expressed as lower bounds on the first-alive window per (core, k) —
    # per-core slot maps are increasing, so later windows follow.
    lb_sigma = {}
    for kf, ms in retire:
        for c in range(NC):
            for k in range(kf, NCHUNK):
                if fk[c][k] is not None:
                    j = off[c] + fk[c][k]
                    lb_sigma[j] = max(lb_sigma.get(j, 0), ms)
    cvec = np.zeros(nv)
    cvec[iL:iL + NCHUNK] = -1
    cvec[iH:iH + NCHUNK] = 1
    rows, ub = [], []
    for c in range(NC):
        for j in range(1, ncore[c]):
            rows.append([(off[c] + j - 1, 1.0), (off[c] + j, -1.0)])
            ub.append(-1.0)
    for c in range(NC):
        for k in range(NCHUNK):
            if fk[c][k] is None:
                continue
            rows.append([(iL + k, 1.0), (off[c] + fk[c][k], -1.0)])
            ub.append(0.0)
            rows.append([(off[c] + lk[c][k], 1.0), (iH + k, -1.0)])
            ub.append(-1.0)
    A = lil_matrix((len(rows), nv))
    for i, coeffs in enumerate(rows):
        for col, v in coeffs:
            A[i, col] = v
    bounds = [(lb_sigma.get(i, 0), NSLOT - 1) for i in range(nS)] + \
        [(0, NSLOT)] * (2 * NCHUNK)
    res = linprog(cvec, A_ub=A.tocsr(), b_ub=np.array(ub), bounds=bounds,
                  method="highs")
    if not res.success:
        raise RuntimeError(f"slot LP failed: {res.message}")
    slots = []
    for c in range(NC):
        arr = np.round(res.x[off[c]:off[c + 1]]).astype(np.int64)
        for j in range(1, ncore[c]):
            if arr[j] <= arr[j - 1]:
                arr[j] = arr[j - 1] + 1
        arr = np.minimum(arr, NSLOT - 1)
        for j in reversed(range(ncore[c] - 1)):
            if arr[j] >= arr[j + 1]:
                arr[j] = arr[j + 1] - 1
        if ncore[c] and arr[0] < 0:
            raise RuntimeError("slot rounding failed")
        slots.append(arr)
    return slots


def _assign_slots_cdf(groups, NSLOT):
    """Fallback: global-CDF slot assignment (v2 scheme)."""
    NC = len(groups)
    entries = [(lo, hi, c, j)
               for c in range(NC)
               for j, (lo, hi, _, _) in enumerate(groups[c])]
    entries.sort(key=lambda t: (t[0], t[1]))
    G_tot = len(entries)
    tgt = [np.zeros(len(groups[c])) for c in range(NC)]
    for r, (lo, hi, c, j) in enumerate(entries):
        tgt[c][j] = (r + 0.5) * NSLOT / G_tot
    slots = []
    for c in range(NC):
        n = len(groups[c])
        arr = np.zeros(n, dtype=np.int64)
        prev = -1
        for j in range(n):
            v = max(prev + 1, int(tgt[c][j]))
            arr[j] = v
            prev = v
        nxt = NSLOT
        for j in reversed(range(n)):
            v = min(int(arr[j]), nxt - 1)
            arr[j] = v
            nxt = v
        slots.append(arr)
    return slots


def _host_prep(features, rois):
    """Build per-core packed device inputs + unpack metadata."""
    R = rois.shape[0]
    batch = rois[:, 0].astype(np.int32)
    x1 = rois[:, 1].astype(np.float64) * SCALE
    y1 = rois[:, 2].astype(np.float64) * SCALE
    x2 = rois[:, 3].astype(np.float64) * SCALE
    y2 = rois[:, 4].astype(np.float64) * SCALE
    bw = (x2 - x1) / POOLED
    bh = (y2 - y1) / POOLED
    pw = np.arange(POOLED, dtype=np.float64)
    xs = x1[:, None] + pw * bw[:, None]
    ys = y1[:, None] + pw * bh[:, None]
    Ix = _tent_integral(xs, xs + bw[:, None], W)       # [R,7,W]
    Iy = _tent_integral(ys, ys + bh[:, None], H)       # [R,7,H]
    area = bw * bh
    scl = np.where(area > 0, 1.0 / np.maximum(area, 1e-12), 0.0)
    Iy_s = Iy * scl[:, None, None]

    # trim tent-tail rows carrying <1% of a window's weight: windows span
    # fewer h-chunks, shrinking the shipped basis ~7% for ~4e-3 rel error
    # (budget 2e-2; bf16 alone costs 2.7e-3).
    TAU = 1e-2
    absIy = np.abs(Iy_s)
    keep = absIy >= TAU * absIy.sum(axis=2, keepdims=True)
    idx = np.arange(H)
    first = np.where(keep.any(2), np.argmax(keep, axis=2), 0)
    last = np.where(keep.any(2), H - 1 - np.argmax(keep[:, :, ::-1], axis=2), 0)
    edge = (idx[None, None, :] < first[:, :, None]) | \
           (idx[None, None, :] > last[:, :, None])
    Iy_s = np.where(edge, 0.0, Iy_s)

    core_rois = [np.nonzero(batch == c)[0] for c in range(NCORES)]
    Rmax = max(len(ix) for ix in core_rois)
    NSLOT = min((4 * BANK) // POOLED, Rmax * POOLED + 36)
    COLS = NSLOT * POOLED
    NBANK = (COLS + BANK - 1) // BANK

    # real group windows per core, sorted by (lo, hi)
    groups = []                                        # [core][(lo,hi,rg,p)]
    for c in range(NCORES):
        wins = []
        for rg in core_rois[c]:
            for p in range(POOLED):
                nz = np.nonzero(Iy_s[rg, p] != 0)[0]
                lo, hi = (int(nz[0]), int(nz[-1])) if len(nz) else (0, 0)
                wins.append((lo, hi, int(rg), p))
        wins.sort(key=lambda t: (t[0], t[1]))
        groups.append(wins)

    spans = [[(lo // CHUNK_H, hi // CHUNK_H) for (lo, hi, _, _) in groups[c]]
             for c in range(NCORES)]
    # retirement targets: banks 0..2 stop being written ~43/61/75% through
    # the chunk sweep (slot bound = ceil(512*(bk+1)/POOLED)).
    retire = tuple((int(round(f * NCHUNK)), (BANK * (bk + 1) + POOLED - 1) // POOLED)
                   for bk, f in enumerate((0.43, 0.61, 0.75)))
    try:
        slots = _assign_slots_lp(spans, NSLOT, retire)
    except Exception:
        try:
            slots = _assign_slots_lp(spans, NSLOT)
        except Exception:
            slots = _assign_slots_cdf(groups, NSLOT)

    # per-chunk alive slot interval (min/max over cores); slack slots that
    # fall inside no interval are simply never written (garbage, ignored).
    LO = np.full(NCHUNK, NSLOT, dtype=np.int64)
    HI = np.full(NCHUNK, -1, dtype=np.int64)
    for c in range(NCORES):
        for j, (a, b) in enumerate(spans[c]):
            s = slots[c][j]
            for k in range(a, b + 1):
                LO[k] = min(LO[k], s)
                HI[k] = max(HI[k], s + 1)
    active = HI >= 0
    LOc, HIc = LO * POOLED, HI * POOLED

    # chunk processing order (ascending h; reorderings measured worse on HW)
    korder = [k for k in range(NCHUNK) if active[k]]
    offs = {}                                  # B block start per chunk
    P = 0
    for k in korder:
        offs[k] = P
        P += int(HIc[k] - LOc[k])
    NB = P

    # pack B per core: B[(dh,w), packed_col]  (112 payload rows, no pad)
    B = np.zeros((NCORES, KDIM, NB), dtype=np.float32)
    IxT = Ix.transpose(0, 2, 1)                        # [R, W, 7]
    for c in range(NCORES):
        for j, (lo, hi, rg, p) in enumerate(groups[c]):
            s = int(slots[c][j])
            for k in range(lo // CHUNK_H, hi // CHUNK_H + 1):
                cb = int(offs[k]) + s * POOLED - int(LOc[k])
                for dh in range(CHUNK_H):
                    h = CHUNK_H * k + dh
                    if lo <= h <= hi:
                        B[c, dh * W:(dh + 1) * W, cb:cb + POOLED] = (
                            Iy_s[rg, p, h] * IxT[rg]
                        )
    # Ship the 16 PE pad rows (K 112->128) as DRAM zeros: the SDMA port
    # swizzle gives partitions 112-127 only the 4 least-loaded ports, so the
    # extra rows do not extend the stream (time ~ columns x 8 rows/port),
    # while K=128 matmuls are ~1.33x faster per column than K=112.
    B = np.pad(B, ((0, 0), (0, KPAD - KDIM), (0, 0))).astype(BF16)

    # features per core, chunk-major transposed: FT[(dh,w), k*C + cc]
    f = features.astype(np.float32)                    # [N,C,H,W]
    ft = f.reshape(N, C, NCHUNK, CHUNK_H, W).transpose(0, 3, 4, 2, 1)
    FT = np.pad(ft.reshape(N, KDIM, NCHUNK * C),
                ((0, 0), (0, KPAD - KDIM), (0, 0))).astype(BF16)

    # merged input image: per split s, [ft cols | B cols] of its chunks,
    # contiguous, so one DMA per split moves one fat descriptor per
    # partition row.  Splits partition korder balanced by column count.
    # split boundaries by cumulative column fraction; measured best with
    # few, fat splits (DMA triggers cost ~0.6us each on sync and the Tile
    # runtime throttles outstanding DMAs with a FIFO semaphore chain, so
    # more/finer splits slow the stream more than they help the tensor
    # engine track the tail).  The last two splits are kept small so the
    # final matmuls + bank-3 evacuation start early.
    FRACS = (0.16, 0.36, 0.56, 0.76, 0.92)
    ncols = [C + int(HIc[k] - LOc[k]) for k in korder]
    tot_cols = sum(ncols)
    bounds, acc, fi = [0], 0, 0
    for i, k in enumerate(korder):
        acc += ncols[i]
        if fi < len(FRACS) and acc >= FRACS[fi] * tot_cols:
            bounds.append(i + 1)
            fi += 1
    bounds.append(len(korder))
    bounds = sorted(set(bounds))
    n_head = len(bounds) - 1

    pieces, ft_off, b_off, split_cols, P = [], {}, {}, [], 0
    for s in range(len(bounds) - 1):
        ks = korder[bounds[s]:bounds[s + 1]]
        a0 = P
        for k in ks:
            pieces.append(FT[:, :, k * C:(k + 1) * C])
            ft_off[k] = P
            P += C
        for k in ks:
            w = int(HIc[k] - LOc[k])
            pieces.append(B[:, :, offs[k]:offs[k] + w])
            b_off[k] = P
            P += w
        split_cols.append((a0, P))
    IN = np.concatenate(pieces, axis=2)
    assert IN.shape == (NCORES, KPAD, P) and P == len(korder) * C + NB

    return dict(IN=IN, korder=korder, LOc=LOc.astype(int), HIc=HIc.astype(int),
                groups=groups, slots=slots, NSLOT=NSLOT, n_head=n_head,
                ft_off=ft_off, b_off=b_off, split_cols=split_cols, TOT=P,
                Rmax=Rmax, COLS=COLS, NBANK=NBANK, NB=NB, R=R)


def _build_bass(shape_key):
    """Build + compile the SPMD Bass program for given packing metadata."""
    (NB, COLS, NBANK, LOc, HIc, korder_t, ft_off_t, b_off_t,
     split_cols_t, TOT, n_head) = shape_key
    LOc, HIc, korder = list(LOc), list(HIc), list(korder_t)
    ft_off, b_off = dict(ft_off_t), dict(b_off_t)
    split_cols = list(split_cols_t)

    import concourse.bass as bass  # noqa: F401
    import concourse.tile as tile
    from concourse import bacc, mybir

    nc = bacc.Bacc("TRN2", target_bir_lowering=False, debug=False,
                   enable_asserts=False, num_devices=NCORES)
    bf = mybir.dt.bfloat16
    f32 = mybir.dt.float32
    in_ap = nc.dram_tensor("inp", [KPAD, TOT], bf, kind="ExternalInput").ap()
    # output is bank-interleaved across the two c-halves so each store piece
    # is ONE contiguous DMA covering both halves: cols of bank bk, half m sit
    # at [2*512*bk + m*w_bk, ...); the host de-interleaves.
    out_ap = nc.dram_tensor("out", [128, 2 * COLS], bf,
                            kind="ExternalOutput").ap()

    # last chunk (in processing order) touching each bank → stop flag;
    # matmuls may not cross a PSUM bank boundary (invalid ISA), so pieces
    # split per bank.
    last_k = {}
    for k in korder:
        for bk in range(LOc[k] // BANK, (HIc[k] - 1) // BANK + 1):
            last_k[bk] = k

    with tile.TileContext(nc) as tc:
        with (
            tc.tile_pool(name="inp", bufs=1) as inp,
            tc.tile_pool(name="pp", bufs=8, space="PSUM") as pp,
            tc.tile_pool(name="op", bufs=2) as op,
        ):
            in_sb = inp.tile([KPAD, TOT], bf)
            # one fat DMA per split (ft+B merged columns): one ~5KB
            # descriptor per partition row, all triggered from sync.
            in_trigs = []
            for a, bnd in split_cols:
                in_trigs.append(
                    nc.sync.dma_start(in_sb[:, a:bnd], in_ap[:, a:bnd]))

            ptiles = [[pp.tile([128, BANK], f32, tag="bank", name=f"pt{m}_{i}")
                       for i in range(NBANK)] for m in range(2)]
            out_sb = op.tile([128, 2 * COLS], bf, name="os")
            # cols written so far per bank (has_written high-water mark);
            # -1 = bank untouched.  The bank's first matmul sets start=True,
            # clearing the whole bank's has_written bits; later writes to
            # fresh columns overwrite, repeat writes accumulate.  Both
            # c-halves run inside the chunk loop so the tensor engine tracks
            # the B stream instead of queueing half the work behind the last
            # input split.
            whi = [[-1] * NBANK for _ in range(2)]
            for k in korder:
                lo, hi = LOc[k], HIc[k]
                fo, bo = ft_off[k], b_off[k]
                for m in range(2):
                    lhsT = in_sb[:, fo + m * 128: fo + (m + 1) * 128]
                    for bk in range(lo // BANK, (hi - 1) // BANK + 1):
                        s = max(lo, bk * BANK)
                        e = min(hi, (bk + 1) * BANK)
                        nc.tensor.matmul(
                            ptiles[m][bk][:, s - bk * BANK: e - bk * BANK],
                            lhsT=lhsT,
                            rhs=in_sb[:, bo + s - lo: bo + e - lo],
                            start=whi[m][bk] < 0,
                            stop=k == last_k[bk],
                        )
                        whi[m][bk] = max(whi[m][bk], e)
            # evacuate each PSUM bank as it retires: half0 on vector, half1
            # on scalar (gpsimd cannot access PSUM); BOTH last-bank copies on
            # vector (it frees first after the last matmul).  Each bank is
            # one contiguous store piece over both interleaved halves,
            # triggered from sync as soon as its copies land.
            for bk in range(NBANK):
                w = min(BANK, COLS - bk * BANK)
                base = 2 * bk * BANK
                for m in range(2):
                    dst = out_sb[:, base + m * w: base + (m + 1) * w]
                    if m == 0:
                        nc.vector.tensor_copy(dst, ptiles[m][bk][:, :w])
                    else:
                        nc.scalar.copy(dst, ptiles[m][bk][:, :w])
                a = 2 * bk * BANK
                e = min(2 * (bk + 1) * BANK, 2 * COLS)
                nc.sync.dma_start(out_ap[:, a:e], out_sb[:, a:e])

    # Splice the first two input triggers ahead of the framework's init
    # barrier (right after sync's engine preamble): the input stream starts
    # ~1us earlier, while other engines are still in their preambles.
    # Dependencies are unaffected — each dma_start has its own completion
    # semaphore that the consuming matmuls wait on.
    main = nc.m.functions[0].blocks[0]
    idx = main.instructions.index(nc.sync.preamble_end) + 1
    for trig in in_trigs[:2]:
        for b in nc.m.functions[0].blocks:
            if trig.ins in b.instructions:
                b.instructions.remove(trig.ins)
                main.instructions.insert(idx, trig.ins)
                idx += 1
                break

    nc.compile()
    return nc


def _ensure_ntff_hook():
    """Some images lack antenv.axon_hooks; recreate it so a BASS_TRACE=1
    environment degrades to (or succeeds at) profiling instead of crashing."""
    import sys
    try:
        import antenv.axon_hooks  # noqa: F401
        return
    except ImportError:
        pass
    try:
        import types
        import antenv
        mod = types.ModuleType("antenv.axon_hooks")
        _hook = [None]
        mod.set_axon_ntff_profile_hook = lambda h: _hook.__setitem__(0, h)
        mod.get_axon_ntff_profile_hook = lambda: _hook[0]
        sys.modules["antenv.axon_hooks"] = mod
        antenv.axon_hooks = mod
        from trn_agent_boot.trn_boot import _ntff_profile_via_ctypes
        mod.set_axon_ntff_profile_hook(
            _ntff_profile_via_ctypes("/opt/axon/libaxon_pjrt.so"))
    except Exception:
        pass


def _unpack(res, hp):
    """out_core[c_chan, col(slot,q)] -> final[r, c_chan, p, q]"""
    COLS, NBANK = hp["COLS"], hp["NBANK"]
    final = np.zeros((hp["R"], C, POOLED, POOLED), dtype=np.float32)
    for c in range(NCORES):
        raw = np.asarray(res.results[c]["out"]).astype(np.float32)  # [128, 2C]
        out = np.zeros((C, COLS), dtype=np.float32)
        for bk in range(NBANK):
            w = min(BANK, COLS - bk * BANK)
            base = 2 * bk * BANK
            for m in range(2):
                out[m * 128:(m + 1) * 128, bk * BANK: bk * BANK + w] = (
                    raw[:, base + m * w: base + (m + 1) * w])
        gs = hp["groups"][c]
        if not gs:
            continue
        rgs = np.array([g[2] for g in gs])
        ps = np.array([g[3] for g in gs])
        sl = np.asarray(hp["slots"][c], dtype=np.int64)
        cols = out.reshape(C, -1, POOLED)[:, sl, :]     # [C, ngrp, 7]
        final[rgs, :, ps, :] = cols.transpose(1, 0, 2)
    return final


def kernel(features, rois):
    global LAST_RESULTS
    _ensure_ntff_hook()
    from concourse import bass_utils

    features = np.asarray(features, dtype=np.float32)
    rois = np.asarray(rois, dtype=np.float32)
    hp = _host_prep(features, rois)

    shape_key = (hp["NB"], hp["COLS"], hp["NBANK"],
                 tuple(hp["LOc"]), tuple(hp["HIc"]),
                 tuple(hp["korder"]),
                 tuple(sorted(hp["ft_off"].items())),
                 tuple(sorted(hp["b_off"].items())),
                 tuple(hp["split_cols"]),
                 int(hp["TOT"]), int(hp["n_head"]))
    nc = _kernel_cache.get(shape_key)
    if nc is None:
        nc = _build_bass(shape_key)
        _kernel_cache[shape_key] = nc

    in_maps = [{"inp": np.ascontiguousarray(hp["IN"][c])}
               for c in range(NCORES)]
    # flaky-device insurance: a wedged core occasionally returns NaN — rerun.
    final = None
    for attempt in range(3):
        res = bass_utils.run_bass_kernel_spmd(nc, in_maps,
                                              core_ids=list(range(NCORES)))
        LAST_RESULTS = res
        final = _unpack(res, hp)
        if np.isfinite(final).all():
            break
    return final


# revision 25
# speedup vs baseline: 1.1016x; 1.1016x over previous
"""PrRoIPool2D (precise ROI pooling) Trainium2 kernel — 8-core SPMD.

Strategy ("fused banded sweep", v3):
  out[r,c,p,q] = sum_{h,w} F[b_r,c,h,w] * Iy[r,p,h] * Ix[r,q,w]
The (Iy ⊗ Ix) basis is banded: bin (r,p) touches only a ~4-row window of h.
Each core owns one feature batch.  Host packs a basis tensor B whose columns
are (r,p,q) output septets; for each 2-row h-chunk k the alive columns form
one contiguous interval [LO_k, HI_k).  The device runs one matmul per
(chunk, c-half, psum-bank-piece) with the features as stationary weights,
PSUM-accumulating straight into the final output columns.

v3+ over v2 (each measured on HW, min/median of 5 — run variance ~1.5us):
  * Slot assignment solved as an LP (per-core increasing slot maps, shared
    per-chunk staircase intervals, minimize total interval width) with slot
    slack NSLOT=288 > Rmax*7: interval columns drop ~30%.  Ties broken
    toward early bank retirement via per-sigma lower bounds (free).
  * No full-coverage chain fix: PSUM `start` clears the whole bank's
    has_written bits, so never-written slack columns just read back garbage
    that the host ignores.
  * Tent tails carrying <1% of a window's weight are trimmed (rel err
    2.7e-3 -> 4.9e-3, budget 2e-2): ~7% fewer basis columns.
  * The 16 zero pad rows (K 112->128) ARE shipped from DRAM: SDMA port
    swizzle gives partitions 112-127 only underloaded ports, so they cost
    no stream time, while K=128 matmuls are ~1.33x faster per column than
    K=112 (cost model claims K-independence; HW disagrees).
  * First two input DMA triggers spliced ahead of the framework's init
    barrier (data starts ~7.5us instead of ~8.3us).
  * Six input splits with a small final one; all triggers on sync in
    emission order (the Tile runtime throttles outstanding DMAs with a
    FIFO semaphore chain — cross-engine or too-fine triggers reorder or
    pace the wire and slow the stream; 7/8-way splits measured no better).
  * Output stored per retired PSUM bank (4 pieces), evacuation split
    vector (half 0) / scalar (half 1) so the final bank drains in one hop.
"""

import numpy as np
import ml_dtypes

POOLED = 7
SCALE = 0.5
N, C, H, W = 8, 256, 56, 56
NCORES = 8
CHUNK_H = 2
NCHUNK = H // CHUNK_H          # 28
KDIM = CHUNK_H * W             # 112 (payload K rows; DMAed)
KPAD = 128                     # PE-array K (pad rows zeroed on device)
BANK = 512                     # fp32 elements per PSUM bank
BF16 = ml_dtypes.bfloat16

_kernel_cache = {}
LAST_RESULTS = None            # BassKernelResults stash for test harnesses


def _tent_integral(start, end, n):
    i = np.arange(n, dtype=np.float64)
    a = np.clip(start[..., None] - i, -1.0, 1.0)
    b = np.clip(end[..., None] - i, -1.0, 1.0)

    def G(t):
        return np.where(t <= 0.0, 0.5 * (t + 1.0) ** 2, 1.0 - 0.5 * (1.0 - t) ** 2)

    return G(b) - G(a)


def _assign_slots_lp(spans, NSLOT, retire=()):
    """Per-core increasing slot maps minimizing total staircase width.

    spans[c] = [(a, b)] chunk spans per window, sorted by (lo, hi).
    Returns slots[c] (np.int64 arrays).  LP: vars sigma (slots), L_k, H_k;
    min sum(H-L) s.t. sigma increasing per core, L_k <= sigma(first alive),
    H_k >= sigma(last alive)+1.  Constraint matrix is network-like, so the
    relaxation is integral in practice; rounding is repaired monotone.
    """
    from scipy.optimize import linprog
    from scipy.sparse import lil_matrix

    NC = len(spans)
    ncore = [len(s) for s in spans]
    fk = [[None] * NCHUNK for _ in range(NC)]
    lk = [[None] * NCHUNK for _ in range(NC)]
    for c in range(NC):
        for j, (a, b) in enumerate(spans[c]):
            for k in range(a, b + 1):
                if fk[c][k] is None:
                    fk[c][k] = j
                lk[c][k] = j
    off = np.cumsum([0] + ncore)
    nS = off[-1]
    iL, iH = nS, nS + NCHUNK
    nv = nS + 2 * NCHUNK
    # bank-retirement: windows alive at chunk >= kf must sit at slot >= ms
    # so low PSUM banks retire (and store) while the stream still runs.
    # Expressed as lower bounds on the first-alive window per (core, k) —
    # per-core slot maps are increasing, so later windows follow.
    lb_sigma = {}
    for kf, ms in retire:
        for c in range(NC):
            for k in range(kf, NCHUNK):
                if fk[c][k] is not None:
                    j = off[c] + fk[c][k]
                    lb_sigma[j] = max(lb_sigma.get(j, 0), ms)
    cvec = np.zeros(nv)
    cvec[iL:iL + NCHUNK] = -1
    cvec[iH:iH + NCHUNK] = 1
    rows, ub = [], []
    for c in range(NC):
        for j in range(1, ncore[c]):
            rows.append([(off[c] + j - 1, 1.0), (off[c] + j, -1.0)])
            ub.append(-1.0)
    for c in range(NC):
        for k in range(NCHUNK):
            if fk[c][k] is None:
                continue
            rows.append([(iL + k, 1.0), (off[c] + fk[c][k], -1.0)])
            ub.append(0.0)
            rows.append([(off[c] + lk[c][k], 1.0), (iH + k, -1.0)])
            ub.append(-1.0)
    A = lil_matrix((len(rows), nv))
    for i, coeffs in enumerate(rows):
        for col, v in coeffs:
            A[i, col] = v
    bounds = [(lb_sigma.get(i, 0), NSLOT - 1) for i in range(nS)] + \
        [(0, NSLOT)] * (2 * NCHUNK)
    res = linprog(cvec, A_ub=A.tocsr(), b_ub=np.array(ub), bounds=bounds,
                  method="highs")
    if not res.success:
        raise RuntimeError(f"slot LP failed: {res.message}")
    slots = []
    for c in range(NC):
        arr = np.round(res.x[off[c]:off[c + 1]]).astype(np.int64)
        for j in range(1, ncore[c]):
            if arr[j] <= arr[j - 1]:
                arr[j] = arr[j - 1] + 1
        arr = np.minimum(arr, NSLOT - 1)
        for j in reversed(range(ncore[c] - 1)):
            if arr[j] >= arr[j + 1]:
                arr[j] = arr[j + 1] - 1
        if ncore[c] and arr[0] < 0:
            raise RuntimeError("slot rounding failed")
        slots.append(arr)
    return slots


def _assign_slots_cdf(groups, NSLOT):
    """Fallback: global-CDF slot assignment (v2 scheme)."""
    NC = len(groups)
    entries = [(lo, hi, c, j)
               for c in range(NC)
               for j, (lo, hi, _, _) in enumerate(groups[c])]
    entries.sort(key=lambda t: (t[0], t[1]))
    G_tot = len(entries)
    tgt = [np.zeros(len(groups[c])) for c in range(NC)]
    for r, (lo, hi, c, j) in enumerate(entries):
        tgt[c][j] = (r + 0.5) * NSLOT / G_tot
    slots = []
    for c in range(NC):
        n = len(groups[c])
        arr = np.zeros(n, dtype=np.int64)
        prev = -1
        for j in range(n):
            v = max(prev + 1, int(tgt[c][j]))
            arr[j] = v
            prev = v
        nxt = NSLOT
        for j in reversed(range(n)):
            v = min(int(arr[j]), nxt - 1)
            arr[j] = v
            nxt = v
        slots.append(arr)
    return slots


def _host_prep(features, rois):
    """Build per-core packed device inputs + unpack metadata."""
    R = rois.shape[0]
    batch = rois[:, 0].astype(np.int32)
    x1 = rois[:, 1].astype(np.float64) * SCALE
    y1 = rois[:, 2].astype(np.float64) * SCALE
    x2 = rois[:, 3].astype(np.float64) * SCALE
    y2 = rois[:, 4].astype(np.float64) * SCALE
    bw = (x2 - x1) / POOLED
    bh = (y2 - y1) / POOLED
    pw = np.arange(POOLED, dtype=np.float64)
    xs = x1[:, None] + pw * bw[:, None]
    ys = y1[:, None] + pw * bh[:, None]
    Ix = _tent_integral(xs, xs + bw[:, None], W)       # [R,7,W]
    Iy = _tent_integral(ys, ys + bh[:, None], H)       # [R,7,H]
    area = bw * bh
    scl = np.where(area > 0, 1.0 / np.maximum(area, 1e-12), 0.0)
    Iy_s = Iy * scl[:, None, None]

    # trim tent-tail rows carrying <1% of a window's weight: windows span
    # fewer h-chunks, shrinking the shipped basis ~7% for ~4e-3 rel error
    # (budget 2e-2; bf16 alone costs 2.7e-3).
    TAU = 1e-2
    absIy = np.abs(Iy_s)
    keep = absIy >= TAU * absIy.sum(axis=2, keepdims=True)
    idx = np.arange(H)
    first = np.where(keep.any(2), np.argmax(keep, axis=2), 0)
    last = np.where(keep.any(2), H - 1 - np.argmax(keep[:, :, ::-1], axis=2), 0)
    edge = (idx[None, None, :] < first[:, :, None]) | \
           (idx[None, None, :] > last[:, :, None])
    Iy_s = np.where(edge, 0.0, Iy_s)

    core_rois = [np.nonzero(batch == c)[0] for c in range(NCORES)]
    Rmax = max(len(ix) for ix in core_rois)
    NSLOT = min((4 * BANK) // POOLED, Rmax * POOLED + 36)
    COLS = NSLOT * POOLED
    NBANK = (COLS + BANK - 1) // BANK

    # real group windows per core, sorted by (lo, hi)
    groups = []                                        # [core][(lo,hi,rg,p)]
    for c in range(NCORES):
        wins = []
        for rg in core_rois[c]:
            for p in range(POOLED):
                nz = np.nonzero(Iy_s[rg, p] != 0)[0]
                lo, hi = (int(nz[0]), int(nz[-1])) if len(nz) else (0, 0)
                wins.append((lo, hi, int(rg), p))
        wins.sort(key=lambda t: (t[0], t[1]))
        groups.append(wins)

    spans = [[(lo // CHUNK_H, hi // CHUNK_H) for (lo, hi, _, _) in groups[c]]
             for c in range(NCORES)]
    # retirement targets: banks 0..2 stop being written ~43/61/75% through
    # the chunk sweep (slot bound = ceil(512*(bk+1)/POOLED)).
    retire = tuple((int(round(f * NCHUNK)), (BANK * (bk + 1) + POOLED - 1) // POOLED)
                   for bk, f in enumerate((0.43, 0.61, 0.75)))
    try:
        slots = _assign_slots_lp(spans, NSLOT, retire)
    except Exception:
        try:
            slots = _assign_slots_lp(spans, NSLOT)
        except Exception:
            slots = _assign_slots_cdf(groups, NSLOT)

    # per-chunk alive slot interval (min/max over cores); slack slots that
    # fall inside no interval are simply never written (garbage, ignored).
    LO = np.full(NCHUNK, NSLOT, dtype=np.int64)
    HI = np.full(NCHUNK, -1, dtype=np.int64)
    for c in range(NCORES):
        for j, (a, b) in enumerate(spans[c]):
            s = slots[c][j]
            for k in range(a, b + 1):
                LO[k] = min(LO[k], s)
                HI[k] = max(HI[k], s + 1)
    active = HI >= 0
    LOc, HIc = LO * POOLED, HI * POOLED

    # chunk processing order (ascending h; reorderings measured worse on HW)
    korder = [k for k in range(NCHUNK) if active[k]]
    offs = {}                                  # B block start per chunk
    P = 0
    for k in korder:
        offs[k] = P
        P += int(HIc[k] - LOc[k])
    NB = P

    # pack B per core: B[(dh,w), packed_col]  (112 payload rows, no pad)
    B = np.zeros((NCORES, KDIM, NB), dtype=np.float32)
    IxT = Ix.transpose(0, 2, 1)                        # [R, W, 7]
    for c in range(NCORES):
        for j, (lo, hi, rg, p) in enumerate(groups[c]):
            s = int(slots[c][j])
            for k in range(lo // CHUNK_H, hi // CHUNK_H + 1):
                cb = int(offs[k]) + s * POOLED - int(LOc[k])
                for dh in range(CHUNK_H):
                    h = CHUNK_H * k + dh
                    if lo <= h <= hi:
                        B[c, dh * W:(dh + 1) * W, cb:cb + POOLED] = (
                            Iy_s[rg, p, h] * IxT[rg]
                        )
    # Ship the 16 PE pad rows (K 112->128) as DRAM zeros: the SDMA port
    # swizzle gives partitions 112-127 only the 4 least-loaded ports, so the
    # extra rows do not extend the stream (time ~ columns x 8 rows/port),
    # while K=128 matmuls are ~1.33x faster per column than K=112.
    B = np.pad(B, ((0, 0), (0, KPAD - KDIM), (0, 0))).astype(BF16)

    # features per core, chunk-major transposed: FT[(dh,w), k*C + cc]
    f = features.astype(np.float32)                    # [N,C,H,W]
    ft = f.reshape(N, C, NCHUNK, CHUNK_H, W).transpose(0, 3, 4, 2, 1)
    FT = np.pad(ft.reshape(N, KDIM, NCHUNK * C),
                ((0, 0), (0, KPAD - KDIM), (0, 0))).astype(BF16)

    # merged input image: per split s, [ft cols | B cols] of its chunks,
    # contiguous, so one DMA per split moves one fat descriptor per
    # partition row.  Splits partition korder balanced by column count.
    # split boundaries by cumulative column fraction; measured best with
    # few, fat splits (DMA triggers cost ~0.6us each on sync and the Tile
    # runtime throttles outstanding DMAs with a FIFO semaphore chain, so
    # more/finer splits slow the stream more than they help the tensor
    # engine track the tail).  The last two splits are kept small so the
    # final matmuls + bank-3 evacuation start early.
    FRACS = (0.16, 0.36, 0.56, 0.76, 0.92)
    ncols = [C + int(HIc[k] - LOc[k]) for k in korder]
    tot_cols = sum(ncols)
    bounds, acc, fi = [0], 0, 0
    for i, k in enumerate(korder):
        acc += ncols[i]
        if fi < len(FRACS) and acc >= FRACS[fi] * tot_cols:
            bounds.append(i + 1)
            fi += 1
    bounds.append(len(korder))
    bounds = sorted(set(bounds))
    n_head = len(bounds) - 1

    pieces, ft_off, b_off, split_cols, P = [], {}, {}, [], 0
    for s in range(len(bounds) - 1):
        ks = korder[bounds[s]:bounds[s + 1]]
        a0 = P
        for k in ks:
            pieces.append(FT[:, :, k * C:(k + 1) * C])
            ft_off[k] = P
            P += C
        for k in ks:
            w = int(HIc[k] - LOc[k])
            pieces.append(B[:, :, offs[k]:offs[k] + w])
            b_off[k] = P
            P += w
        split_cols.append((a0, P))
    IN = np.concatenate(pieces, axis=2)
    assert IN.shape == (NCORES, KPAD, P) and P == len(korder) * C + NB

    return dict(IN=IN, korder=korder, LOc=LOc.astype(int), HIc=HIc.astype(int),
                groups=groups, slots=slots, NSLOT=NSLOT, n_head=n_head,
                ft_off=ft_off, b_off=b_off, split_cols=split_cols, TOT=P,
                Rmax=Rmax, COLS=COLS, NBANK=NBANK, NB=NB, R=R)


def _build_bass(shape_key):
    """Build + compile the SPMD Bass program for given packing metadata."""
    (NB, COLS, NBANK, LOc, HIc, korder_t, ft_off_t, b_off_t,
     split_cols_t, TOT, n_head) = shape_key
    LOc, HIc, korder = list(LOc), list(HIc), list(korder_t)
    ft_off, b_off = dict(ft_off_t), dict(b_off_t)
    split_cols = list(split_cols_t)

    import concourse.bass as bass  # noqa: F401
    import concourse.tile as tile
    from concourse import bacc, mybir

    nc = bacc.Bacc("TRN2", target_bir_lowering=False, debug=False,
                   enable_asserts=False, num_devices=NCORES)
    bf = mybir.dt.bfloat16
    f32 = mybir.dt.float32
    in_ap = nc.dram_tensor("inp", [KPAD, TOT], bf, kind="ExternalInput").ap()
    # output is bank-interleaved across the two c-halves so each store piece
    # is ONE contiguous DMA covering both halves: cols of bank bk, half m sit
    # at [2*512*bk + m*w_bk, ...); the host de-interleaves.
    out_ap = nc.dram_tensor("out", [128, 2 * COLS], bf,
                            kind="ExternalOutput").ap()

    # last chunk (in processing order) touching each bank → stop flag;
    # matmuls may not cross a PSUM bank boundary (invalid ISA), so pieces
    # split per bank.
    last_k = {}
    for k in korder:
        for bk in range(LOc[k] // BANK, (HIc[k] - 1) // BANK + 1):
            last_k[bk] = k

    with tile.TileContext(nc) as tc:
        with (
            tc.tile_pool(name="inp", bufs=1) as inp,
            tc.tile_pool(name="pp", bufs=8, space="PSUM") as pp,
            tc.tile_pool(name="op", bufs=2) as op,
        ):
            in_sb = inp.tile([KPAD, TOT], bf)
            # one fat DMA per split (ft+B merged columns): one ~5KB
            # descriptor per partition row, all triggered from sync.
            in_trigs = []
            for a, bnd in split_cols:
                in_trigs.append(
                    nc.sync.dma_start(in_sb[:, a:bnd], in_ap[:, a:bnd]))

            ptiles = [[pp.tile([128, BANK], f32, tag="bank", name=f"pt{m}_{i}")
                       for i in range(NBANK)] for m in range(2)]
            # PE p-state warm-up: ~3us of dummy matmuls during the preamble
            # /early stream (the PE reaches max clock only after ~3us of
            # continuous execution; the real stream's semaphore gaps never
            # get it there from cold).  Dummies write PSUM bank 0, which
            # the first real matmul clears via start=True.
            warm = inp.tile([128, 128], bf, name="warm")
            nc.gpsimd.memset(warm[:, :], 0.0)
            for _ in range(35):
                nc.tensor.matmul(ptiles[0][0][:, 0:128], lhsT=warm[:, :],
                                 rhs=warm[:, :], start=True, stop=True)
            out_sb = op.tile([128, 2 * COLS], bf, name="os")
            # cols written so far per bank (has_written high-water mark);
            # -1 = bank untouched.  The bank's first matmul sets start=True,
            # clearing the whole bank's has_written bits; later writes to
            # fresh columns overwrite, repeat writes accumulate.  Both
            # c-halves run inside the chunk loop so the tensor engine tracks
            # the B stream instead of queueing half the work behind the last
            # input split.
            whi = [[-1] * NBANK for _ in range(2)]
            for k in korder:
                lo, hi = LOc[k], HIc[k]
                fo, bo = ft_off[k], b_off[k]
                for m in range(2):
                    lhsT = in_sb[:, fo + m * 128: fo + (m + 1) * 128]
                    for bk in range(lo // BANK, (hi - 1) // BANK + 1):
                        s = max(lo, bk * BANK)
                        e = min(hi, (bk + 1) * BANK)
                        nc.tensor.matmul(
                            ptiles[m][bk][:, s - bk * BANK: e - bk * BANK],
                            lhsT=lhsT,
                            rhs=in_sb[:, bo + s - lo: bo + e - lo],
                            start=whi[m][bk] < 0,
                            stop=k == last_k[bk],
                        )
                        whi[m][bk] = max(whi[m][bk], e)
            # evacuate each PSUM bank as it retires: half0 on vector, half1
            # on scalar (gpsimd cannot access PSUM); BOTH last-bank copies on
            # vector (it frees first after the last matmul).  Each bank is
            # one contiguous store piece over both interleaved halves,
            # triggered from sync as soon as its copies land.
            for bk in range(NBANK):
                w = min(BANK, COLS - bk * BANK)
                base = 2 * bk * BANK
                for m in range(2):
                    dst = out_sb[:, base + m * w: base + (m + 1) * w]
                    if m == 0:
                        nc.vector.tensor_copy(dst, ptiles[m][bk][:, :w])
                    else:
                        nc.scalar.copy(dst, ptiles[m][bk][:, :w])
                a = 2 * bk * BANK
                e = min(2 * (bk + 1) * BANK, 2 * COLS)
                nc.sync.dma_start(out_ap[:, a:e], out_sb[:, a:e])

    # Splice the first two input triggers ahead of the framework's init
    # barrier (right after sync's engine preamble): the input stream starts
    # ~1us earlier, while other engines are still in their preambles.
    # Dependencies are unaffected — each dma_start has its own completion
    # semaphore that the consuming matmuls wait on.
    main = nc.m.functions[0].blocks[0]
    idx = main.instructions.index(nc.sync.preamble_end) + 1
    for trig in in_trigs[:2]:
        for b in nc.m.functions[0].blocks:
            if trig.ins in b.instructions:
                b.instructions.remove(trig.ins)
                main.instructions.insert(idx, trig.ins)
                idx += 1
                break

    nc.compile()
    return nc


def _ensure_ntff_hook():
    """Some images lack antenv.axon_hooks; recreate it so a BASS_TRACE=1
    environment degrades to (or succeeds at) profiling instead of crashing."""
    import sys
    try:
        import antenv.axon_hooks  # noqa: F401
        return
    except ImportError:
        pass
    try:
        import types
        import antenv
        mod = types.ModuleType("antenv.axon_hooks")
        _hook = [None]
        mod.set_axon_ntff_profile_hook = lambda h: _hook.__setitem__(0, h)
        mod.get_axon_ntff_profile_hook = lambda: _hook[0]
        sys.modules["antenv.axon_hooks"] = mod
        antenv.axon_hooks = mod
        from trn_agent_boot.trn_boot import _ntff_profile_via_ctypes
        mod.set_axon_ntff_profile_hook(
            _ntff_profile_via_ctypes("/opt/axon/libaxon_pjrt.so"))
    except Exception:
        pass


def _unpack(res, hp):
    """out_core[c_chan, col(slot,q)] -> final[r, c_chan, p, q]"""
    COLS, NBANK = hp["COLS"], hp["NBANK"]
    final = np.zeros((hp["R"], C, POOLED, POOLED), dtype=np.float32)
    for c in range(NCORES):
        raw = np.asarray(res.results[c]["out"]).astype(np.float32)  # [128, 2C]
        out = np.zeros((C, COLS), dtype=np.float32)
        for bk in range(NBANK):
            w = min(BANK, COLS - bk * BANK)
            base = 2 * bk * BANK
            for m in range(2):
                out[m * 128:(m + 1) * 128, bk * BANK: bk * BANK + w] = (
                    raw[:, base + m * w: base + (m + 1) * w])
        gs = hp["groups"][c]
        if not gs:
            continue
        rgs = np.array([g[2] for g in gs])
        ps = np.array([g[3] for g in gs])
        sl = np.asarray(hp["slots"][c], dtype=np.int64)
        cols = out.reshape(C, -1, POOLED)[:, sl, :]     # [C, ngrp, 7]
        final[rgs, :, ps, :] = cols.transpose(1, 0, 2)
    return final


def kernel(features, rois):
    global LAST_RESULTS
    _ensure_ntff_hook()
    from concourse import bass_utils

    features = np.asarray(features, dtype=np.float32)
    rois = np.asarray(rois, dtype=np.float32)
    hp = _host_prep(features, rois)

    shape_key = (hp["NB"], hp["COLS"], hp["NBANK"],
                 tuple(hp["LOc"]), tuple(hp["HIc"]),
                 tuple(hp["korder"]),
                 tuple(sorted(hp["ft_off"].items())),
                 tuple(sorted(hp["b_off"].items())),
                 tuple(hp["split_cols"]),
                 int(hp["TOT"]), int(hp["n_head"]))
    nc = _kernel_cache.get(shape_key)
    if nc is None:
        nc = _build_bass(shape_key)
        _kernel_cache[shape_key] = nc

    in_maps = [{"inp": np.ascontiguousarray(hp["IN"][c])}
               for c in range(NCORES)]
    # flaky-device insurance: a wedged core occasionally returns NaN — rerun.
    final = None
    for attempt in range(3):
        res = bass_utils.run_bass_kernel_spmd(nc, in_maps,
                                              core_ids=list(range(NCORES)))
        LAST_RESULTS = res
        final = _unpack(res, hp)
        if np.isfinite(final).all():
            break
    return final
